# revision 42
# baseline (speedup 1.0000x reference)
"""Trainium2 Bass kernel for GQA attention layer (B=1, S=2048, H=4096,
32 Q heads / 8 KV heads, head_dim 128, RoPE with arbitrary tables).

Sharding: tensor-parallel over heads across 8 NeuronCores — core c gets
Q heads 4c..4c+3 and KV head c (Wq/Wk/Wv column shards, Wo row shard).
Each core computes its partial o_proj output [2048, 4096]; the host sums
the 8 partials (equivalent of the all-reduce).

Per-core compute (all matmuls bf16 with fp32 PSUM accumulation), run as a
software-pipelined fusion of three phases so the PE never idles behind the
ACT-paced attention:
  Phase A: qT/kT/vT = W.T @ hs.T in [d, s] layout (N=512 matmuls), RoPE via
           rotate-half partition-swap DMA + DVE ops; v transposed to [s, d]
           chunks on the DMA XBAR.
  Phase B: flash-style causal attention per (head, q-range, k-tile pair):
           scoresT[k,q] = kT.T @ qT into a [128,1024] PSUM pair tile so one
           ACT exp covers two k-tiles (amortizes the 352-cycle ACTIVATE
           overhead); attn_oT[d,q] += v[k,d].T @ probsT; diagonal k-tiles
           column-narrowed + one triangular mask multiply. Denominator:
           bf16 DVE pair-folds -> one bf16 [128,1]x[128,512] PE matmul ->
           fast reciprocal -> gpsimd partition_broadcast -> fused DVE mul.
  Phase C: partial o_proj [s, hidden] = attn_oT.T @ Wo_shard, staged into
           [128, 4096] row blocks (8KB output DMA descriptors).
  Schedule: S(qr) = B(qr) with A(qr+1)'s projection matmuls emitted as PE
           filler inside B's exp-wait slots (quotas sized per q-range), so
           attention's ACT latency hides under projection work; B(3) uses
           o_proj matmuls as filler and the remainder drains at the end.
           DMA queue discipline matters: hst/wo prefetches are front-run on
           the sync queue, rope swaps stay off the scalar FIFO (they would
           block exps), and startup transfers are chunked in consumption
           order so the first matmul starts ~1MB into the input stream.
"""

import sys
from contextlib import ExitStack

sys.path.insert(0, "/opt/trn_rl_repo")

import numpy as np
import ml_dtypes

import concourse.bacc as bacc
import concourse.mybir as mybir
import concourse.tile as tile
from concourse.bass_utils import run_bass_kernel_spmd

BF16 = mybir.dt.bfloat16
F32 = mybir.dt.float32

N_CORES = 8
S = 2048
HID = 4096
D = 128
NQ = 4  # q heads per core
KC = HID // 128  # 32 hidden-dim chunks
NQR = S // 512  # 4 q ranges of 512
NST = S // 128  # 16 s-tiles of 128
NHO = HID // 512  # 8 output column tiles of 512
SCALE = 1.0 / float(np.sqrt(D))

_CACHE: dict = {}


def _build_nc():
    nc = bacc.Bacc(None, target_bir_lowering=False, debug=False)

    hst_d = nc.dram_tensor("hst", [NQR, 128, KC, 512], BF16, kind="ExternalInput")
    wq_d = nc.dram_tensor("wq", [NQ, 128, KC, D], BF16, kind="ExternalInput")
    wk_d = nc.dram_tensor("wk", [128, KC, D], BF16, kind="ExternalInput")
    wv_d = nc.dram_tensor("wv", [128, KC, D], BF16, kind="ExternalInput")
    wo_d = nc.dram_tensor("wo", [128, NQ, HID], BF16, kind="ExternalInput")
    cos_d = nc.dram_tensor("cos2", [128, S], F32, kind="ExternalInput")
    sin_d = nc.dram_tensor("sin2", [128, S], F32, kind="ExternalInput")
    # Partials leave the core as bf16: each core's o_proj partial is summed
    # with 7 others on the host in fp32, so the extra bf16 rounding is ~0.1%
    # in quadrature — and the output DMA bytes halve (33.5MB -> 16.8MB).
    out_d = nc.dram_tensor("out", [S, HID], BF16, kind="ExternalOutput")

    with tile.TileContext(nc) as tc, ExitStack() as stack:
        # ---- pools that live the whole kernel ----
        const = stack.enter_context(tc.tile_pool(name="const", bufs=1))
        act = stack.enter_context(tc.tile_pool(name="act", bufs=1))
        qt_sb = [
            act.tile([128, S], BF16, tag=f"qt{h}", name=f"qt{h}") for h in range(NQ)
        ]
        kt_sb = act.tile([128, S], BF16, tag="kt")
        vt_sb = act.tile([128, S], BF16, tag="vt")
        v_sb = act.tile([128, NST, 128], BF16, tag="v")  # [s,d] chunks per k-tile
        attn_sb = [
            act.tile([128, S], BF16, tag=f"attn{h}", name=f"attn{h}")
            for h in range(NQ)
        ]
        probs_p = stack.enter_context(tc.tile_pool(name="probs", bufs=3))
        den_p = stack.enter_context(tc.tile_pool(name="den", bufs=2))
        bcast_p = stack.enter_context(tc.tile_pool(name="bcast", bufs=2))
        # PSUM: psS scores pairs 2x[128,1024] = 4 banks + psO 2 banks are
        # reserved first; psA (phase-A accumulators, 2 banks) lives only
        # until attention range 3 — the o_proj "c" accumulators (2 banks)
        # allocate from the space psA frees.
        psS = stack.enter_context(tc.tile_pool(name="psS", bufs=2, space="PSUM"))
        psO = stack.enter_context(tc.tile_pool(name="psO", bufs=2, space="PSUM"))

        # ================== o_proj (phase C) unit machinery ==================
        # wo_sb is created late (SBUF is full while the A pools live); the
        # closures below resolve it at call time.
        wo_ref = {}

        def c_units():
            for qrC in range(NQR):
                for st in range(qrC * 4, qrC * 4 + 4):
                    for ho in range(NHO):
                        yield ("alloc", qrC, st, ho)
                        for h in range(NQ):
                            yield ("mm", qrC, st, ho, h)
                        yield ("evict", qrC, st, ho)

        c_state = {"gen": c_units(), "pending": None, "tile": None,
                   "row": None, "evict_alt": 0}

        def emit_c(n_mms, qr_done, evict_engine="v"):
            emitted = 0
            while emitted < n_mms:
                unit = c_state["pending"] or next(c_state["gen"], None)
                c_state["pending"] = None
                if unit is None:
                    return False
                if unit[1] > qr_done:
                    c_state["pending"] = unit
                    return False
                if unit[0] == "alloc":
                    _, _, st, ho = unit
                    c_state["tile"] = c_state["psC"].tile(
                        [128, 512], F32, tag="c", bufs=2, name=f"c{st}_{ho}"
                    )
                elif unit[0] == "mm":
                    _, _, st, ho, h = unit
                    nc.tensor.matmul(
                        c_state["tile"][:],
                        attn_sb[h][:, st * 128 : (st + 1) * 128],
                        wo_ref["wo"][:, h, ho * 512 : (ho + 1) * 512],
                        start=(h == 0),
                        stop=(h == NQ - 1),
                        skip_group_check=True,
                    )
                    emitted += 1
                else:
                    _, _, st, ho = unit
                    # Stage into a [128, 4096] row-block, one DMA per st:
                    # 8KB output descriptors instead of 1KB ones.
                    if ho == 0:
                        c_state["row"] = c_state["ostage"].tile(
                            [128, HID], BF16, tag="stg", name=f"row{st}"
                        )
                    dst = c_state["row"][:, ho * 512 : (ho + 1) * 512]
                    if evict_engine == "v":
                        nc.vector.tensor_copy(dst, c_state["tile"][:])
                    else:
                        nc.scalar.copy(dst, c_state["tile"][:])
                    if st == NST - 1:
                        # the kernel's very last row: flush each 512-col
                        # slice as it evicts so the final DMA is 128KB, not
                        # a 1MB row transfer serialized after the last copy
                        nc.sync.dma_start(
                            out_d[
                                st * 128 : (st + 1) * 128,
                                ho * 512 : (ho + 1) * 512,
                            ],
                            dst,
                        )
                    elif ho == NHO - 1:
                        nc.sync.dma_start(
                            out_d[st * 128 : (st + 1) * 128, :],
                            c_state["row"][:],
                        )
            return True

        # =========== fused phase A (QKV+RoPE) / phase B (attention) ===========
        with (
            tc.tile_pool(name="wqkv", bufs=1) as wqkv,
            tc.tile_pool(name="hstp", bufs=2) as hstp,
            tc.tile_pool(name="rope", bufs=2) as rope,
            tc.tile_pool(name="psA", bufs=2, space="PSUM") as psA,
        ):
            # Startup DMAs, in consumption order and chunked so the first
            # k-projection matmul only waits for wk chunk 0 + hst chunk 0.
            hst_tiles = []
            wk_sb = wqkv.tile([128, KC, D], BF16)
            wv_sb = wqkv.tile([128, KC, D], BF16)
            hst_t0 = hstp.tile([128, KC, 512], BF16, tag="hst", name="hst0")
            # finest chunks first (the k matmul chain starts after ~0.4MB),
            # wv interleaved so the v job (at ~21us) never waits
            chunks = [(0, 2), (2, 2), (4, 2), (6, 2)] + [
                (c, 4) for c in range(8, KC, 4)
            ]
            for c, w in chunks:
                nc.sync.dma_start(
                    wk_sb[:, c : c + w, :], wk_d[:, c : c + w, :]
                )
                nc.sync.dma_start(
                    hst_t0[:, c : c + w, :], hst_d[0, :, c : c + w, :]
                )
                nc.sync.dma_start(
                    wv_sb[:, c : c + w, :], wv_d[:, c : c + w, :]
                )
            hst_tiles.append(hst_t0)
            # rope tables: only the qr=0 column slice is needed early (the
            # k-projection's rope evict at ~21us); the rest can trail wq.
            cos_sb = const.tile([128, S], F32)
            sin_sb = const.tile([128, S], F32)
            nc.sync.dma_start(cos_sb[:, 0:512], cos_d[:, 0:512])
            nc.sync.dma_start(sin_sb[:, 0:512], sin_d[:, 0:512])
            wq_sb = [
                wqkv.tile([128, KC, D], BF16, tag=f"wq{h}", name=f"wq{h}")
                for h in range(NQ)
            ]
            for h in range(NQ):
                for r in range(2):
                    nc.sync.dma_start(
                        wq_sb[h][:, r * 16 : (r + 1) * 16, :],
                        wq_d[h, :, r * 16 : (r + 1) * 16, :],
                    )
            for r in range(1, 4):
                nc.sync.dma_start(
                    cos_sb[:, r * 512 : (r + 1) * 512],
                    cos_d[:, r * 512 : (r + 1) * 512],
                )
                nc.sync.dma_start(
                    sin_sb[:, r * 512 : (r + 1) * 512],
                    sin_d[:, r * 512 : (r + 1) * 512],
                )

            ones = const.tile([128, 1], BF16)
            nc.gpsimd.memset(ones[:], 1.0)
            # triangular mask for the diagonal 128x128 subtile: rows are k,
            # cols are q; keep q >= k.
            tri = const.tile([128, 128], BF16)
            nc.gpsimd.memset(tri[:], 1.0)
            nc.gpsimd.affine_select(
                out=tri[:],
                in_=tri[:],
                pattern=[[1, 128]],
                compare_op=mybir.AluOpType.is_ge,
                fill=0.0,
                base=0,
                channel_multiplier=-1,
            )
            # Preload the ACT engine's Exp table while it is idle.
            warm = const.tile([1, 2], BF16)
            nc.scalar.activation(
                warm[:], tri[0:1, 0:2],
                mybir.ActivationFunctionType.Exp, scale=1.0,
            )

            def rope_evict(ps, dst_tile, qr):
                """dst[0:64]  = x0*cos - x1*sin
                dst[64:128] = x1*cos + x0*sin   (x0=ps[0:64], x1=ps[64:128])
                The partition swap stays on the sync queue: a DMA submission
                in the scalar FIFO that is still waiting on its DVE input
                would block the exps queued behind it (strict FIFO)."""
                sl = slice(qr * 512, (qr + 1) * 512)
                raw = rope.tile([128, 512], F32, tag="raw")
                nc.vector.tensor_copy(raw[:], ps[:])
                swp = rope.tile([128, 512], F32, tag="swp")
                nc.sync.dma_start(swp[0:64, :], raw[64:128, :])
                nc.sync.dma_start(swp[64:128, :], raw[0:64, :])
                nc.vector.tensor_mul(raw[:], raw[:], cos_sb[:, sl])
                nc.vector.tensor_mul(swp[:], swp[:], sin_sb[:, sl])
                nc.vector.tensor_sub(dst_tile[0:64, sl], raw[0:64, :], swp[0:64, :])
                nc.vector.tensor_add(
                    dst_tile[64:128, sl], raw[64:128, :], swp[64:128, :]
                )

            # ---- phase-A unit stream: one yield per projection matmul, with
            # alloc/evict/transpose/prefetch units interleaved at no PE cost.
            def a_units(qr):
                if qr >= NQR:
                    return
                if 1 <= qr < NQR - 1:
                    # prefetch hst[qr+1]: its buffer was freed when A(qr-1)
                    # finished, one full S-phase ago.
                    yield ("pref", qr + 1)
                jobs = [("k", 0), ("v", 0)] + [("q", h) for h in range(NQ)]
                for kind, h in jobs:
                    yield ("alloc", qr, kind, h)
                    for c in range(KC):
                        yield ("mm", qr, kind, h, c)
                    yield ("evict", qr, kind, h)
                    if kind == "v":
                        # transpose right after vT lands: the XBAR transfer
                        # gets the whole remaining S-phase to finish instead
                        # of bunching right before B(qr) consumes v_sb.
                        for kt in range(qr * 4, qr * 4 + 4):
                            yield ("vtrans", kt)
                if qr == 0:
                    # for qr=0 the prefetch is issued after the jobs so the
                    # startup burst keeps ring priority.
                    yield ("pref", 1)

            a_state = {"gen": None, "ps": None}

            def do_pref(nqr):
                nxt = hstp.tile(
                    [128, KC, 512], BF16, tag="hst", name=f"hst{nqr}"
                )
                for r in range(4):
                    nc.sync.dma_start(
                        nxt[:, r * 8 : (r + 1) * 8, :],
                        hst_d[nqr, :, r * 8 : (r + 1) * 8, :],
                    )
                hst_tiles.append(nxt)

            def emit_a(n_mms):
                """Emit up to n_mms projection matmuls (plus any free units
                hit along the way). Returns #mms actually emitted."""
                emitted = 0
                gen = a_state["gen"]
                if gen is None:
                    return 0
                while emitted < n_mms:
                    unit = next(gen, None)
                    if unit is None:
                        a_state["gen"] = None
                        break
                    if unit[0] == "pref":
                        do_pref(unit[1])
                    elif unit[0] == "alloc":
                        _, qr, kind, h = unit
                        a_state["ps"] = psA.tile(
                            [128, 512], F32, tag="a", name=f"a{qr}_{kind}{h}"
                        )
                    elif unit[0] == "mm":
                        _, qr, kind, h, c = unit
                        if kind == "q":
                            lhsT = wq_sb[h][:, c, :]
                        elif kind == "k":
                            lhsT = wk_sb[:, c, :]
                        else:
                            lhsT = wv_sb[:, c, :]
                        nc.tensor.matmul(
                            a_state["ps"][:],
                            lhsT,
                            hst_tiles[qr][:, c, :],
                            start=(c == 0),
                            stop=(c == KC - 1),
                        )
                        emitted += 1
                    elif unit[0] == "evict":
                        _, qr, kind, h = unit
                        if kind == "q":
                            rope_evict(a_state["ps"], qt_sb[h], qr)
                        elif kind == "k":
                            rope_evict(a_state["ps"], kt_sb, qr)
                        else:
                            sl = slice(qr * 512, (qr + 1) * 512)
                            nc.vector.tensor_copy(
                                vt_sb[:, sl], a_state["ps"][:]
                            )
                    else:  # vtrans — DMA_TRANSPOSE occupies its queue engine
                        # for ~1.2us, so keep it OFF the scalar queue (which
                        # runs the exps) and early in the sync queue.
                        kt = unit[1]
                        nc.sync.dma_start_transpose(
                            v_sb[:, kt, :], vt_sb[:, kt * 128 : (kt + 1) * 128]
                        )
                return emitted

            def fill_a_then_c(n, qr):
                # wo isn't resident until the A pools close, so o_proj can't
                # act as backup filler here; quotas are sized so the A
                # stream outlasts each S-phase's slots anyway.
                emit_a(n)

            # ---- attention head body (phase B) with pluggable PE filler ----
            def b_head(qr, h, fill, q_fill):
                n_kt = 4 * (qr + 1)
                n_pair = n_kt // 2
                qsl = slice(qr * 512, (qr + 1) * 512)

                def c0_of(kt):
                    p_idx = kt - 4 * qr
                    return 128 * p_idx if p_idx > 0 else 0

                ps_o = psO.tile([128, 512], F32, tag="o", name=f"o{qr}_{h}")
                den_acc = den_p.tile(
                    [128, 512], BF16, tag="da", name=f"da{qr}_{h}"
                )
                pair_tiles = {}

                def mm_scores_pair(j):
                    ps_s = psS.tile(
                        [128, 1024], F32, tag="s", name=f"s{qr}_{h}_{j}"
                    )
                    for idx in range(2):
                        kt = 2 * j + idx
                        c0 = c0_of(kt)
                        nc.tensor.matmul(
                            ps_s[:, idx * 512 + c0 : (idx + 1) * 512],
                            kt_sb[:, kt * 128 : (kt + 1) * 128],
                            qt_sb[h][:, qr * 512 + c0 : (qr + 1) * 512],
                            start=True,
                            stop=True,
                        )
                    pair_tiles[j] = ps_s

                mm_scores_pair(0)
                ps_s = pair_tiles[0]
                for j in range(n_pair):
                    if j + 1 < n_pair:
                        mm_scores_pair(j + 1)
                    is_diag = (2 * j + 1) >= 4 * qr
                    fill(q_fill * (2 if is_diag else 1), qr)
                    ps_s = pair_tiles.pop(j)
                    pt = probs_p.tile(
                        [128, 1024], BF16, tag="pt", name=f"pt{qr}_{h}_{j}"
                    )
                    if 2 * j + 1 < 4 * qr:
                        nc.scalar.activation(
                            pt[:],
                            ps_s[:],
                            mybir.ActivationFunctionType.Exp,
                            scale=SCALE,
                        )
                    else:
                        for idx in range(2):
                            kt = 2 * j + idx
                            c0 = c0_of(kt)
                            nc.scalar.activation(
                                pt[:, idx * 512 + c0 : (idx + 1) * 512],
                                ps_s[:, idx * 512 + c0 : (idx + 1) * 512],
                                mybir.ActivationFunctionType.Exp,
                                scale=SCALE,
                            )
                            nc.vector.tensor_mul(
                                pt[:, idx * 512 + c0 : idx * 512 + c0 + 128],
                                pt[:, idx * 512 + c0 : idx * 512 + c0 + 128],
                                tri[:],
                            )
                    for idx in range(2):
                        kt = 2 * j + idx
                        c0 = c0_of(kt)
                        nc.tensor.matmul(
                            ps_o[:, c0:512],
                            v_sb[:, kt, :],
                            pt[:, idx * 512 + c0 : (idx + 1) * 512],
                            start=(kt == 0),
                            stop=(kt == n_kt - 1),
                            skip_group_check=True,
                        )
                    pts = probs_p.tile(
                        [128, 512], BF16, tag="pts", name=f"pts{qr}_{h}_{j}"
                    )
                    c0a, c0b = c0_of(2 * j), c0_of(2 * j + 1)
                    if c0b > c0a:
                        nc.vector.tensor_copy(pts[:, c0a:c0b], pt[:, c0a:c0b])
                    nc.vector.tensor_add(
                        pts[:, c0b:512],
                        pt[:, c0b:512],
                        pt[:, 512 + c0b : 1024],
                    )
                    if j == 0:
                        nc.vector.tensor_copy(den_acc[:], pts[:])
                    else:
                        nc.vector.tensor_add(
                            den_acc[:, c0a:512],
                            den_acc[:, c0a:512],
                            pts[:, c0a:512],
                        )
                fill(q_fill, qr)
                # Partition-reduce den_acc with one bf16 matmul into the
                # (dead) last scores psum tile, then recip + broadcast +
                # fused normalize.
                nc.tensor.matmul(
                    ps_s[0:1, 0:512],
                    ones[:],
                    den_acc[:],
                    start=True,
                    stop=True,
                    skip_group_check=True,
                )
                recip = den_p.tile([1, 512], F32, tag="recip", name=f"rc{qr}_{h}")
                nc.vector.reciprocal_approx_fast(
                    out=recip[:], in_=ps_s[0:1, 0:512]
                )
                bc = bcast_p.tile([128, 512], F32, tag="bc")
                nc.gpsimd.partition_broadcast(bc[:], recip[:])
                nc.vector.tensor_mul(attn_sb[h][:, qsl], ps_o[:], bc[:])

            # ---- A(0) runs dense, then S(qr) = B(qr) + A(qr+1) as filler ----
            a_state["gen"] = a_units(0)
            while emit_a(10**9):
                pass
            # Per-slot A-matmul filler quota, sized so ~198 projection MMs
            # spread across each S-phase's fill slots (diag slots get 2x):
            # qr0: 20 weighted slots, qr1: 28, qr2: 36.
            quota = {0: 10, 1: 7, 2: 6}
            for qr in range(NQR - 1):
                a_state["gen"] = a_units(qr + 1)
                for h in range(NQ):
                    b_head(qr, h, fill_a_then_c, quota[qr])
                while emit_a(10**9):
                    pass

        # A pools + psA freed.  wo + output staging allocate in their space;
        # o_proj "c" accumulators take psA's two PSUM banks.
        wo_pool = stack.enter_context(tc.tile_pool(name="wo", bufs=1))
        wo_sb = wo_pool.tile([128, NQ, HID], BF16)
        wo_ref["wo"] = wo_sb
        # column-groups, matching o_proj's ho-inner consumption order, so
        # the first c-filler matmuls only wait for the first ~0.5MB
        for g in range(8):
            nc.sync.dma_start(
                wo_sb[:, :, g * 512 : (g + 1) * 512],
                wo_d[:, :, g * 512 : (g + 1) * 512],
            )
        c_state["ostage"] = stack.enter_context(
            tc.tile_pool(name="ostage", bufs=3)
        )
        # o_proj accumulators land on the two PSUM banks psA just freed
        c_state["psC"] = stack.enter_context(
            tc.tile_pool(name="psC", bufs=2, space="PSUM")
        )

        def fill_c(n, qr):
            emit_c(n, qr - 1)

        for h in range(NQ):
            b_head(NQR - 1, h, fill_c, 4)

        # ---- drain the remaining o_proj work ----
        alt = 0
        while emit_c(4, NQR - 1, evict_engine=("v" if alt % 2 == 0 else "s")):
            alt += 1

    nc.compile()
    return nc


def _get_nc():
    if "nc" not in _CACHE:
        _CACHE["nc"] = _build_nc()
    return _CACHE["nc"]


def _bf16(x):
    return np.ascontiguousarray(x.astype(ml_dtypes.bfloat16))


def _prep_in_maps(hidden_states, sin_table, cos_table, Wq, Wk, Wv, Wo):
    hs0 = np.asarray(hidden_states, np.float32).reshape(S, HID)
    # hst[qr, p, c, s] = hs0[qr*512 + s, c*128 + p]
    hst = _bf16(hs0.reshape(NQR, 512, KC, 128).transpose(0, 3, 2, 1))
    cosT = np.asarray(cos_table, np.float32).T  # [64, S]
    sinT = np.asarray(sin_table, np.float32).T
    cos2 = np.ascontiguousarray(np.concatenate([cosT, cosT], 0))  # [128, S]
    sin2 = np.ascontiguousarray(np.concatenate([sinT, sinT], 0))
    Wq = np.asarray(Wq, np.float32)
    Wk = np.asarray(Wk, np.float32)
    Wv = np.asarray(Wv, np.float32)
    Wo = np.asarray(Wo, np.float32)

    in_maps = []
    for c in range(N_CORES):
        wq_c = Wq[:, c * 512 : (c + 1) * 512]  # 4 q heads
        wk_c = Wk[:, c * 128 : (c + 1) * 128]  # 1 kv head
        wv_c = Wv[:, c * 128 : (c + 1) * 128]
        wo_c = Wo[c * 512 : (c + 1) * 512, :]  # matching rows
        # wq per-head-major: [h, p, c, d] with element Wq_c[c*128+p, h*128+d]
        wq_l = wq_c.reshape(KC, 128, NQ, D).transpose(2, 1, 0, 3)
        in_maps.append(
            {
                "hst": hst,
                "wq": _bf16(wq_l),
                "wk": _bf16(wk_c.reshape(KC, 128, D).swapaxes(0, 1)),
                "wv": _bf16(wv_c.reshape(KC, 128, D).swapaxes(0, 1)),
                "wo": _bf16(wo_c.reshape(NQ, 128, HID).swapaxes(0, 1)),
                "cos2": cos2,
                "sin2": sin2,
            }
        )
    return in_maps


def run(trace=False, **inputs):
    nc = _get_nc()
    in_maps = _prep_in_maps(**inputs)
    res = run_bass_kernel_spmd(
        nc, in_maps, core_ids=list(range(N_CORES)), trace=trace
    )
    partials = np.stack(
        [np.asarray(res.results[c]["out"], np.float32) for c in range(N_CORES)]
    )
    out = partials.sum(axis=0, dtype=np.float32).reshape(1, S, HID)
    return out, res


def kernel(**inputs):
    out, _ = run(trace=False, **inputs)
    return out



# revision 43
# speedup vs baseline: 1.1532x; 1.1532x over previous
"""Trainium2 Bass kernel for GQA attention layer (B=1, S=2048, H=4096,
32 Q heads / 8 KV heads, head_dim 128, RoPE with arbitrary tables).

Sharding: tensor-parallel over heads across 8 NeuronCores — core c gets
Q heads 4c..4c+3 and KV head c (Wq/Wk/Wv column shards, Wo row shard).
Each core computes its partial o_proj output [2048, 4096]; the host sums
the 8 partials (equivalent of the all-reduce).

Per-core compute (all matmuls bf16 with fp32 PSUM accumulation), run as a
software-pipelined fusion of three phases so the PE never idles behind the
ACT-paced attention:
  Phase A: qT/kT/vT = W.T @ hs.T in [d, s] layout (N=512 matmuls), RoPE via
           rotate-half partition-swap DMA + DVE ops; v transposed to [s, d]
           chunks on the DMA XBAR.
  Phase B: flash-style causal attention per (head, q-range, k-tile pair):
           scoresT[k,q] = kT.T @ qT into a [128,1024] PSUM pair tile so one
           ACT exp covers two k-tiles (amortizes the 352-cycle ACTIVATE
           overhead); attn_oT[d,q] += v[k,d].T @ probsT; diagonal k-tiles
           column-narrowed + one triangular mask multiply. Denominator:
           bf16 DVE pair-folds -> one bf16 [128,1]x[128,512] PE matmul ->
           fast reciprocal -> gpsimd partition_broadcast -> fused DVE mul.
  Phase C: partial o_proj [s, hidden] = attn_oT.T @ Wo_shard, staged into
           [128, 4096] row blocks (8KB output DMA descriptors).
  Schedule: S(qr) = B(qr) with A(qr+1)'s projection matmuls emitted as PE
           filler inside B's exp-wait slots (quotas sized per q-range), so
           attention's ACT latency hides under projection work; B(3) uses
           o_proj matmuls as filler and the remainder drains at the end.
           DMA queue discipline matters: hst/wo prefetches are front-run on
           the sync queue, rope swaps stay off the scalar FIFO (they would
           block exps), and startup transfers are chunked in consumption
           order so the first matmul starts ~1MB into the input stream.
"""

import sys
from contextlib import ExitStack

sys.path.insert(0, "/opt/trn_rl_repo")

import numpy as np
import ml_dtypes

import concourse.bacc as bacc
import concourse.mybir as mybir
import concourse.tile as tile
from concourse.bass_utils import run_bass_kernel_spmd

BF16 = mybir.dt.bfloat16
F32 = mybir.dt.float32

N_CORES = 8
S = 2048
HID = 4096
D = 128
NQ = 4  # q heads per core
KC = HID // 128  # 32 hidden-dim chunks
NQR = S // 512  # 4 q ranges of 512
NST = S // 128  # 16 s-tiles of 128
NHO = HID // 512  # 8 output column tiles of 512
SCALE = 1.0 / float(np.sqrt(D))

_CACHE: dict = {}


def _build_nc():
    nc = bacc.Bacc(None, target_bir_lowering=False, debug=False)

    hst_d = nc.dram_tensor("hst", [NQR, 128, KC, 512], BF16, kind="ExternalInput")
    wq_d = nc.dram_tensor("wq", [NQ, 128, KC, D], BF16, kind="ExternalInput")
    wk_d = nc.dram_tensor("wk", [128, KC, D], BF16, kind="ExternalInput")
    wv_d = nc.dram_tensor("wv", [128, KC, D], BF16, kind="ExternalInput")
    wo_d = nc.dram_tensor("wo", [128, NQ, HID], BF16, kind="ExternalInput")
    cos_d = nc.dram_tensor("cos2", [128, S], F32, kind="ExternalInput")
    sin_d = nc.dram_tensor("sin2", [128, S], F32, kind="ExternalInput")
    # Partials leave the core as bf16: each core's o_proj partial is summed
    # with 7 others on the host in fp32, so the extra bf16 rounding is ~0.1%
    # in quadrature — and the output DMA bytes halve (33.5MB -> 16.8MB).
    out_d = nc.dram_tensor("out", [S, HID], BF16, kind="ExternalOutput")

    with tile.TileContext(nc) as tc, ExitStack() as stack:
        # ---- pools that live the whole kernel ----
        const = stack.enter_context(tc.tile_pool(name="const", bufs=1))
        act = stack.enter_context(tc.tile_pool(name="act", bufs=1))
        qt_sb = [
            act.tile([128, S], BF16, tag=f"qt{h}", name=f"qt{h}") for h in range(NQ)
        ]
        kt_sb = act.tile([128, S], BF16, tag="kt")
        vt_sb = act.tile([128, S], BF16, tag="vt")
        v_sb = act.tile([128, NST, 128], BF16, tag="v")  # [s,d] chunks per k-tile
        attn_sb = [
            act.tile([128, S], BF16, tag=f"attn{h}", name=f"attn{h}")
            for h in range(NQ)
        ]
        probs_p = stack.enter_context(tc.tile_pool(name="probs", bufs=3))
        den_p = stack.enter_context(tc.tile_pool(name="den", bufs=2))
        bcast_p = stack.enter_context(tc.tile_pool(name="bcast", bufs=2))
        # PSUM: psS scores pairs 2x[128,1024] = 4 banks + psO 2 banks are
        # reserved first; psA (phase-A accumulators, 2 banks) lives only
        # until attention range 3 — the o_proj "c" accumulators (2 banks)
        # allocate from the space psA frees.
        psS = stack.enter_context(tc.tile_pool(name="psS", bufs=2, space="PSUM"))
        psO = stack.enter_context(tc.tile_pool(name="psO", bufs=2, space="PSUM"))

        # ================== o_proj (phase C) unit machinery ==================
        # wo_sb is created late (SBUF is full while the A pools live); the
        # closures below resolve it at call time.
        wo_ref = {}

        def c_units():
            for qrC in range(NQR):
                for st in range(qrC * 4, qrC * 4 + 4):
                    for ho in range(NHO):
                        yield ("alloc", qrC, st, ho)
                        for h in range(NQ):
                            yield ("mm", qrC, st, ho, h)
                        yield ("evict", qrC, st, ho)

        c_state = {"gen": c_units(), "pending": None, "tile": None,
                   "row": None, "evict_alt": 0}

        def emit_c(n_mms, qr_done, evict_engine="v"):
            emitted = 0
            while emitted < n_mms:
                unit = c_state["pending"] or next(c_state["gen"], None)
                c_state["pending"] = None
                if unit is None:
                    return False
                if unit[1] > qr_done:
                    c_state["pending"] = unit
                    return False
                if unit[0] == "alloc":
                    _, _, st, ho = unit
                    c_state["tile"] = c_state["psC"].tile(
                        [128, 512], F32, tag="c", bufs=2, name=f"c{st}_{ho}"
                    )
                elif unit[0] == "mm":
                    _, _, st, ho, h = unit
                    nc.tensor.matmul(
                        c_state["tile"][:],
                        attn_sb[h][:, st * 128 : (st + 1) * 128],
                        wo_ref["wo"][:, h, ho * 512 : (ho + 1) * 512],
                        start=(h == 0),
                        stop=(h == NQ - 1),
                        skip_group_check=True,
                    )
                    emitted += 1
                else:
                    _, _, st, ho = unit
                    # Stage into a [128, 4096] row-block, one DMA per st:
                    # 8KB output descriptors instead of 1KB ones.
                    if ho == 0:
                        c_state["row"] = c_state["ostage"].tile(
                            [128, HID], BF16, tag="stg", name=f"row{st}"
                        )
                    dst = c_state["row"][:, ho * 512 : (ho + 1) * 512]
                    if evict_engine == "v":
                        nc.vector.tensor_copy(dst, c_state["tile"][:])
                    else:
                        nc.scalar.copy(dst, c_state["tile"][:])
                    if st == NST - 1:
                        # the kernel's very last row: flush each 512-col
                        # slice as it evicts so the final DMA is 128KB, not
                        # a 1MB row transfer serialized after the last copy
                        nc.sync.dma_start(
                            out_d[
                                st * 128 : (st + 1) * 128,
                                ho * 512 : (ho + 1) * 512,
                            ],
                            dst,
                        )
                    elif ho == NHO - 1:
                        nc.sync.dma_start(
                            out_d[st * 128 : (st + 1) * 128, :],
                            c_state["row"][:],
                        )
            return True

        # =========== fused phase A (QKV+RoPE) / phase B (attention) ===========
        with (
            tc.tile_pool(name="wqkv", bufs=1) as wqkv,
            tc.tile_pool(name="hstp", bufs=2) as hstp,
            tc.tile_pool(name="rope", bufs=2) as rope,
            tc.tile_pool(name="psA", bufs=2, space="PSUM") as psA,
        ):
            # Startup DMAs, in consumption order and chunked so the first
            # k-projection matmul only waits for wk chunk 0 + hst chunk 0.
            hst_tiles = []
            wk_sb = wqkv.tile([128, KC, D], BF16)
            wv_sb = wqkv.tile([128, KC, D], BF16)
            hst_t0 = hstp.tile([128, KC, 512], BF16, tag="hst", name="hst0")
            # finest chunks first (the k matmul chain starts after ~0.4MB),
            # wv interleaved so the v job (at ~21us) never waits
            chunks = [(0, 2), (2, 2), (4, 2), (6, 2)] + [
                (c, 4) for c in range(8, KC, 4)
            ]
            for c, w in chunks:
                nc.sync.dma_start(
                    wk_sb[:, c : c + w, :], wk_d[:, c : c + w, :]
                )
                nc.sync.dma_start(
                    hst_t0[:, c : c + w, :], hst_d[0, :, c : c + w, :]
                )
                nc.sync.dma_start(
                    wv_sb[:, c : c + w, :], wv_d[:, c : c + w, :]
                )
            hst_tiles.append(hst_t0)
            # rope tables: only the qr=0 column slice is needed early (the
            # k-projection's rope evict at ~21us); the rest can trail wq.
            cos_sb = const.tile([128, S], F32)
            sin_sb = const.tile([128, S], F32)
            nc.sync.dma_start(cos_sb[:, 0:512], cos_d[:, 0:512])
            nc.sync.dma_start(sin_sb[:, 0:512], sin_d[:, 0:512])
            wq_sb = [
                wqkv.tile([128, KC, D], BF16, tag=f"wq{h}", name=f"wq{h}")
                for h in range(NQ)
            ]
            for h in range(NQ):
                for r in range(2):
                    nc.sync.dma_start(
                        wq_sb[h][:, r * 16 : (r + 1) * 16, :],
                        wq_d[h, :, r * 16 : (r + 1) * 16, :],
                    )
            for r in range(1, 4):
                nc.sync.dma_start(
                    cos_sb[:, r * 512 : (r + 1) * 512],
                    cos_d[:, r * 512 : (r + 1) * 512],
                )
                nc.sync.dma_start(
                    sin_sb[:, r * 512 : (r + 1) * 512],
                    sin_d[:, r * 512 : (r + 1) * 512],
                )

            ones = const.tile([128, 1], BF16)
            nc.gpsimd.memset(ones[:], 1.0)
            # triangular mask for the diagonal 128x128 subtile: rows are k,
            # cols are q; keep q >= k.
            tri = const.tile([128, 128], BF16)
            nc.gpsimd.memset(tri[:], 1.0)
            nc.gpsimd.affine_select(
                out=tri[:],
                in_=tri[:],
                pattern=[[1, 128]],
                compare_op=mybir.AluOpType.is_ge,
                fill=0.0,
                base=0,
                channel_multiplier=-1,
            )
            # Preload the ACT engine's Exp table while it is idle.
            warm = const.tile([1, 2], BF16)
            nc.scalar.activation(
                warm[:], tri[0:1, 0:2],
                mybir.ActivationFunctionType.Exp, scale=1.0,
            )

            def rope_evict(ps, dst_tile, qr):
                """dst[0:64]  = x0*cos - x1*sin
                dst[64:128] = x1*cos + x0*sin   (x0=ps[0:64], x1=ps[64:128])
                The partition swap stays on the sync queue: a DMA submission
                in the scalar FIFO that is still waiting on its DVE input
                would block the exps queued behind it (strict FIFO)."""
                sl = slice(qr * 512, (qr + 1) * 512)
                raw = rope.tile([128, 512], F32, tag="raw")
                nc.vector.tensor_copy(raw[:], ps[:])
                swp = rope.tile([128, 512], F32, tag="swp")
                nc.sync.dma_start(swp[0:64, :], raw[64:128, :])
                nc.sync.dma_start(swp[64:128, :], raw[0:64, :])
                nc.vector.tensor_mul(raw[:], raw[:], cos_sb[:, sl])
                nc.vector.tensor_mul(swp[:], swp[:], sin_sb[:, sl])
                nc.vector.tensor_sub(dst_tile[0:64, sl], raw[0:64, :], swp[0:64, :])
                nc.vector.tensor_add(
                    dst_tile[64:128, sl], raw[64:128, :], swp[64:128, :]
                )

            # ---- phase-A unit stream: one yield per projection matmul, with
            # alloc/evict/transpose/prefetch units interleaved at no PE cost.
            def a_units(qr):
                if qr >= NQR:
                    return
                if 1 <= qr < NQR - 1:
                    # prefetch hst[qr+1]: its buffer was freed when A(qr-1)
                    # finished, one full S-phase ago.
                    yield ("pref", qr + 1)
                jobs = [("k", 0), ("v", 0)] + [("q", h) for h in range(NQ)]
                for kind, h in jobs:
                    yield ("alloc", qr, kind, h)
                    for c in range(KC):
                        yield ("mm", qr, kind, h, c)
                    yield ("evict", qr, kind, h)
                    if kind == "q" and h == 1:
                        # v transposes: each DMA_TRANSPOSE holds the sync
                        # sequencer ~1.2us (gated on the vT eviction), so
                        # emit them behind q0/q1's time-critical rope swaps
                        # but still half an S-phase before B(qr) needs v_sb.
                        for kt in range(qr * 4, qr * 4 + 4):
                            yield ("vtrans", kt)
                if qr == 0:
                    # for qr=0 the prefetch is issued after the jobs so the
                    # startup burst keeps ring priority.
                    yield ("pref", 1)

            a_state = {"gen": None, "ps": None}

            def do_pref(nqr):
                nxt = hstp.tile(
                    [128, KC, 512], BF16, tag="hst", name=f"hst{nqr}"
                )
                for r in range(4):
                    nc.sync.dma_start(
                        nxt[:, r * 8 : (r + 1) * 8, :],
                        hst_d[nqr, :, r * 8 : (r + 1) * 8, :],
                    )
                hst_tiles.append(nxt)

            def emit_a(n_mms):
                """Emit up to n_mms projection matmuls (plus any free units
                hit along the way). Returns #mms actually emitted."""
                emitted = 0
                gen = a_state["gen"]
                if gen is None:
                    return 0
                while emitted < n_mms:
                    unit = next(gen, None)
                    if unit is None:
                        a_state["gen"] = None
                        break
                    if unit[0] == "pref":
                        do_pref(unit[1])
                    elif unit[0] == "alloc":
                        _, qr, kind, h = unit
                        a_state["ps"] = psA.tile(
                            [128, 512], F32, tag="a", name=f"a{qr}_{kind}{h}"
                        )
                    elif unit[0] == "mm":
                        _, qr, kind, h, c = unit
                        if kind == "q":
                            lhsT = wq_sb[h][:, c, :]
                        elif kind == "k":
                            lhsT = wk_sb[:, c, :]
                        else:
                            lhsT = wv_sb[:, c, :]
                        nc.tensor.matmul(
                            a_state["ps"][:],
                            lhsT,
                            hst_tiles[qr][:, c, :],
                            start=(c == 0),
                            stop=(c == KC - 1),
                        )
                        emitted += 1
                    elif unit[0] == "evict":
                        _, qr, kind, h = unit
                        if kind == "q":
                            rope_evict(a_state["ps"], qt_sb[h], qr)
                        elif kind == "k":
                            rope_evict(a_state["ps"], kt_sb, qr)
                        else:
                            sl = slice(qr * 512, (qr + 1) * 512)
                            nc.vector.tensor_copy(
                                vt_sb[:, sl], a_state["ps"][:]
                            )
                    else:  # vtrans — DMA_TRANSPOSE occupies its queue engine
                        # for ~1.2us, so keep it OFF the scalar queue (which
                        # runs the exps) and early in the sync queue.
                        kt = unit[1]
                        nc.sync.dma_start_transpose(
                            v_sb[:, kt, :], vt_sb[:, kt * 128 : (kt + 1) * 128]
                        )
                return emitted

            def fill_a_then_c(n, qr):
                # wo isn't resident until the A pools close, so o_proj can't
                # act as backup filler here; quotas are sized so the A
                # stream outlasts each S-phase's slots anyway.
                emit_a(n)

            # ---- attention head body (phase B) with pluggable PE filler ----
            def b_head(qr, h, fill, q_fill):
                n_kt = 4 * (qr + 1)
                n_pair = n_kt // 2
                qsl = slice(qr * 512, (qr + 1) * 512)

                def c0_of(kt):
                    p_idx = kt - 4 * qr
                    return 128 * p_idx if p_idx > 0 else 0

                ps_o = psO.tile([128, 512], F32, tag="o", name=f"o{qr}_{h}")
                den_acc = den_p.tile(
                    [128, 512], BF16, tag="da", name=f"da{qr}_{h}"
                )
                pair_tiles = {}

                def mm_scores_pair(j):
                    ps_s = psS.tile(
                        [128, 1024], F32, tag="s", name=f"s{qr}_{h}_{j}"
                    )
                    for idx in range(2):
                        kt = 2 * j + idx
                        c0 = c0_of(kt)
                        nc.tensor.matmul(
                            ps_s[:, idx * 512 + c0 : (idx + 1) * 512],
                            kt_sb[:, kt * 128 : (kt + 1) * 128],
                            qt_sb[h][:, qr * 512 + c0 : (qr + 1) * 512],
                            start=True,
                            stop=True,
                        )
                    pair_tiles[j] = ps_s

                mm_scores_pair(0)
                ps_s = pair_tiles[0]
                for j in range(n_pair):
                    if j + 1 < n_pair:
                        mm_scores_pair(j + 1)
                    is_diag = (2 * j + 1) >= 4 * qr
                    fill(q_fill * (2 if is_diag else 1), qr)
                    ps_s = pair_tiles.pop(j)
                    pt = probs_p.tile(
                        [128, 1024], BF16, tag="pt", name=f"pt{qr}_{h}_{j}"
                    )
                    if 2 * j + 1 < 4 * qr:
                        nc.scalar.activation(
                            pt[:],
                            ps_s[:],
                            mybir.ActivationFunctionType.Exp,
                            scale=SCALE,
                        )
                    else:
                        for idx in range(2):
                            kt = 2 * j + idx
                            c0 = c0_of(kt)
                            nc.scalar.activation(
                                pt[:, idx * 512 + c0 : (idx + 1) * 512],
                                ps_s[:, idx * 512 + c0 : (idx + 1) * 512],
                                mybir.ActivationFunctionType.Exp,
                                scale=SCALE,
                            )
                            nc.vector.tensor_mul(
                                pt[:, idx * 512 + c0 : idx * 512 + c0 + 128],
                                pt[:, idx * 512 + c0 : idx * 512 + c0 + 128],
                                tri[:],
                            )
                    for idx in range(2):
                        kt = 2 * j + idx
                        c0 = c0_of(kt)
                        nc.tensor.matmul(
                            ps_o[:, c0:512],
                            v_sb[:, kt, :],
                            pt[:, idx * 512 + c0 : (idx + 1) * 512],
                            start=(kt == 0),
                            stop=(kt == n_kt - 1),
                            skip_group_check=True,
                        )
                    pts = probs_p.tile(
                        [128, 512], BF16, tag="pts", name=f"pts{qr}_{h}_{j}"
                    )
                    c0a, c0b = c0_of(2 * j), c0_of(2 * j + 1)
                    if c0b > c0a:
                        nc.vector.tensor_copy(pts[:, c0a:c0b], pt[:, c0a:c0b])
                    nc.vector.tensor_add(
                        pts[:, c0b:512],
                        pt[:, c0b:512],
                        pt[:, 512 + c0b : 1024],
                    )
                    if j == 0:
                        nc.vector.tensor_copy(den_acc[:], pts[:])
                    else:
                        nc.vector.tensor_add(
                            den_acc[:, c0a:512],
                            den_acc[:, c0a:512],
                            pts[:, c0a:512],
                        )
                fill(q_fill, qr)
                # Partition-reduce den_acc with one bf16 matmul into the
                # (dead) last scores psum tile, then recip + broadcast +
                # fused normalize.
                nc.tensor.matmul(
                    ps_s[0:1, 0:512],
                    ones[:],
                    den_acc[:],
                    start=True,
                    stop=True,
                    skip_group_check=True,
                )
                recip = den_p.tile([1, 512], F32, tag="recip", name=f"rc{qr}_{h}")
                nc.vector.reciprocal_approx_fast(
                    out=recip[:], in_=ps_s[0:1, 0:512]
                )
                bc = bcast_p.tile([128, 512], F32, tag="bc")
                nc.gpsimd.partition_broadcast(bc[:], recip[:])
                nc.vector.tensor_mul(attn_sb[h][:, qsl], ps_o[:], bc[:])

            # ---- A(0) runs dense, then S(qr) = B(qr) + A(qr+1) as filler ----
            a_state["gen"] = a_units(0)
            while emit_a(10**9):
                pass
            # Per-slot A-matmul filler quota, sized so ~198 projection MMs
            # spread across each S-phase's fill slots (diag slots get 2x):
            # qr0: 20 weighted slots, qr1: 28, qr2: 36.
            quota = {0: 10, 1: 7, 2: 6}
            for qr in range(NQR - 1):
                a_state["gen"] = a_units(qr + 1)
                for h in range(NQ):
                    b_head(qr, h, fill_a_then_c, quota[qr])
                while emit_a(10**9):
                    pass

        # A pools + psA freed.  wo + output staging allocate in their space;
        # o_proj "c" accumulators take psA's two PSUM banks.
        wo_pool = stack.enter_context(tc.tile_pool(name="wo", bufs=1))
        wo_sb = wo_pool.tile([128, NQ, HID], BF16)
        wo_ref["wo"] = wo_sb
        # column-groups, matching o_proj's ho-inner consumption order, so
        # the first c-filler matmuls only wait for the first ~0.5MB
        for g in range(8):
            nc.sync.dma_start(
                wo_sb[:, :, g * 512 : (g + 1) * 512],
                wo_d[:, :, g * 512 : (g + 1) * 512],
            )
        c_state["ostage"] = stack.enter_context(
            tc.tile_pool(name="ostage", bufs=3)
        )
        # o_proj accumulators land on the two PSUM banks psA just freed
        c_state["psC"] = stack.enter_context(
            tc.tile_pool(name="psC", bufs=2, space="PSUM")
        )

        def fill_c(n, qr):
            emit_c(n, qr - 1)

        for h in range(NQ):
            b_head(NQR - 1, h, fill_c, 4)

        # ---- drain the remaining o_proj work ----
        alt = 0
        while emit_c(4, NQR - 1, evict_engine=("v" if alt % 2 == 0 else "s")):
            alt += 1

    nc.compile()
    return nc


def _get_nc():
    if "nc" not in _CACHE:
        _CACHE["nc"] = _build_nc()
    return _CACHE["nc"]


def _bf16(x):
    return np.ascontiguousarray(x.astype(ml_dtypes.bfloat16))


def _prep_in_maps(hidden_states, sin_table, cos_table, Wq, Wk, Wv, Wo):
    hs0 = np.asarray(hidden_states, np.float32).reshape(S, HID)
    # hst[qr, p, c, s] = hs0[qr*512 + s, c*128 + p]
    hst = _bf16(hs0.reshape(NQR, 512, KC, 128).transpose(0, 3, 2, 1))
    cosT = np.asarray(cos_table, np.float32).T  # [64, S]
    sinT = np.asarray(sin_table, np.float32).T
    cos2 = np.ascontiguousarray(np.concatenate([cosT, cosT], 0))  # [128, S]
    sin2 = np.ascontiguousarray(np.concatenate([sinT, sinT], 0))
    Wq = np.asarray(Wq, np.float32)
    Wk = np.asarray(Wk, np.float32)
    Wv = np.asarray(Wv, np.float32)
    Wo = np.asarray(Wo, np.float32)

    in_maps = []
    for c in range(N_CORES):
        wq_c = Wq[:, c * 512 : (c + 1) * 512]  # 4 q heads
        wk_c = Wk[:, c * 128 : (c + 1) * 128]  # 1 kv head
        wv_c = Wv[:, c * 128 : (c + 1) * 128]
        wo_c = Wo[c * 512 : (c + 1) * 512, :]  # matching rows
        # wq per-head-major: [h, p, c, d] with element Wq_c[c*128+p, h*128+d]
        wq_l = wq_c.reshape(KC, 128, NQ, D).transpose(2, 1, 0, 3)
        in_maps.append(
            {
                "hst": hst,
                "wq": _bf16(wq_l),
                "wk": _bf16(wk_c.reshape(KC, 128, D).swapaxes(0, 1)),
                "wv": _bf16(wv_c.reshape(KC, 128, D).swapaxes(0, 1)),
                "wo": _bf16(wo_c.reshape(NQ, 128, HID).swapaxes(0, 1)),
                "cos2": cos2,
                "sin2": sin2,
            }
        )
    return in_maps


def run(trace=False, **inputs):
    nc = _get_nc()
    in_maps = _prep_in_maps(**inputs)
    res = run_bass_kernel_spmd(
        nc, in_maps, core_ids=list(range(N_CORES)), trace=trace
    )
    partials = np.stack(
        [np.asarray(res.results[c]["out"], np.float32) for c in range(N_CORES)]
    )
    out = partials.sum(axis=0, dtype=np.float32).reshape(1, S, HID)
    return out, res


def kernel(**inputs):
    out, _ = run(trace=False, **inputs)
    return out

